# revision 44
# baseline (speedup 1.0000x reference)
"""Blockwise 8x8 2D DCT (ortho DCT-II) on Trainium2, 8 NeuronCores data-parallel.

Per 8x8 block: Y = A @ X @ A.T, with M = kron(I_16, A) acting on 128-row tiles.

Key trick ("fused" op): a regular PE matmul with the DATA as the stationary
operand computes  out = chunk^T @ M^T = (M @ chunk)^T  — one DCT pass plus a
128x128 transpose in a single instruction. Two fused passes give
  pass1: (M X)^T   (W-major)     pass2: ((M X) M^T)  (back to H-major)
In bf16 the stationary load gets FWL (2x), so each fused op is ~LDW+128 cols.

Modes:
  fused_bf16  : cast x->bf16 at DMA load (gpsimd SWDGE cast); both passes fused bf16.
  hybrid      : V-pass as f32r streaming matmul (x stays fp32-exact), bf16
                transposes, fused bf16 H-pass.
  stream_f32r : f32r streaming matmuls + f32r PE transposes both directions.
"""

import numpy as np
import ml_dtypes

import concourse.bass as bass
import concourse.bacc as bacc
import concourse.mybir as mybir
from concourse import tile
from concourse.bass_utils import run_bass_kernel_spmd

F32 = mybir.dt.float32
F32R = mybir.dt.float32r
BF16 = mybir.dt.bfloat16
FP16 = mybir.dt.float16
P = 128
BLOCK = 8
N_CORES = 8

FULL_N, FULL_C, FULL_H, FULL_W = 64, 1, 1024, 1024

MODE = "v15"


class _CopyBalancer:
    """Deterministically split PSUM->SBUF copies between DVE and ACT."""

    def __init__(self, nc, dve_of_8=5):
        self.nc = nc
        self.k = dve_of_8
        self.i = 0

    def copy(self, out, in_):
        if self.i % 8 < self.k:
            self.nc.vector.tensor_copy(out, in_)
        else:
            self.nc.scalar.copy(out, in_)
        self.i += 1


def build_fused_bf16(n_img: int, img_h: int, width: int, dt16=BF16):
    rows = n_img * img_h
    nrt, nwt = img_h // P, width // P
    assert nrt % 4 == 0 and nwt % 4 == 0

    nc = bacc.Bacc("TRN2", target_bir_lowering=False, debug=False)
    x_d = nc.declare_dram_parameter("x", [rows, width], F32, isOutput=False)
    mtb_d = nc.declare_dram_parameter("mtb", [P, P], dt16, isOutput=False)
    out_d = nc.declare_dram_parameter("out", [rows, width], F32, isOutput=True)

    with tile.TileContext(nc) as tc:
        with (
            tc.tile_pool(name="consts", bufs=1) as cpool,
            tc.tile_pool(name="xin", bufs=nrt + 4) as xpool,
            tc.tile_pool(name="y1t", bufs=nwt + 4) as y1tpool,
            tc.tile_pool(name="outp", bufs=4) as outpool,
            tc.tile_pool(name="psA", bufs=4, space="PSUM") as psA,
            tc.tile_pool(name="psB", bufs=4, space="PSUM") as psB,
        ):
            cb = _CopyBalancer(nc)
            mtb_sb = cpool.tile([P, P], dt16)
            nc.sync.dma_start(mtb_sb[:], mtb_d[:])

            for img in range(n_img):
                r0 = img * img_h

                xts = []
                for rt in range(nrt):
                    xt = xpool.tile([P, width], dt16)
                    # SWDGE cast f32 -> f16 during the load; halves ramp faster
                    for h in range(2):
                        cols = slice(h * width // 2, (h + 1) * width // 2)
                        nc.gpsimd.dma_start(
                            xt[:, cols],
                            x_d[r0 + rt * P : r0 + (rt + 1) * P, cols],
                        )
                    xts.append(xt)

                # pass 1: y1t[wt][:, rt*128:+128] = (M @ x_chunk)^T
                y1ts = []
                for wt in range(nwt):
                    y1t = y1tpool.tile([P, img_h], dt16)
                    for half in range(nrt // 4):
                        ps = psA.tile([P, 512], F32)
                        for q in range(4):
                            rt = half * 4 + q
                            nc.tensor.matmul(
                                ps[:, q * P : (q + 1) * P],
                                xts[rt][:, wt * P : (wt + 1) * P],
                                mtb_sb[:],
                            )
                        cb.copy(y1t[:, half * 512 : (half + 1) * 512], ps[:])
                    y1ts.append(y1t)

                # pass 2: out[rt][:, wt*128:+128] = (M @ y1t_chunk)^T = final
                for rt in range(nrt):
                    out_sb = outpool.tile([P, width], F32)
                    for half in range(nwt // 4):
                        ps = psB.tile([P, 512], F32)
                        for q in range(4):
                            wt = half * 4 + q
                            nc.tensor.matmul(
                                ps[:, q * P : (q + 1) * P],
                                y1ts[wt][:, rt * P : (rt + 1) * P],
                                mtb_sb[:],
                            )
                        cb.copy(out_sb[:, half * 512 : (half + 1) * 512], ps[:])
                        # store each 512-col half as soon as it lands
                        nc.sync.dma_start(
                            out_d[
                                r0 + rt * P : r0 + (rt + 1) * P,
                                half * 512 : (half + 1) * 512,
                            ],
                            out_sb[:, half * 512 : (half + 1) * 512],
                        )

    nc.compile()
    return nc


def build_v2(n_img: int, img_h: int, width: int, dt16=FP16, psum_fd=512,
             act_of_4=2, ps_bufs=8, store_eng="scalar", x_bufs=None):
    """fp16 DRAM in/out (host casts both ways): halves HBM traffic vs f32.

    HWDGE loads on the sync ring, stores on the scalar ring (separate FIFOs).
    Matmuls accumulate f32 into PSUM tiles of [128, psum_fd]; the PSUM->SBUF
    copy casts to fp16 and is split between ACT and DVE (act_of_4 of 4).
    """
    rows = n_img * img_h
    nrt, nwt = img_h // P, width // P
    mm_per_ps = psum_fd // P

    nc = bacc.Bacc("TRN2", target_bir_lowering=False, debug=False)
    x_d = nc.declare_dram_parameter("x", [rows, width], dt16, isOutput=False)
    mtb_d = nc.declare_dram_parameter("mtb", [P, P], dt16, isOutput=False)
    out_d = nc.declare_dram_parameter("out", [rows, width], dt16, isOutput=True)

    if x_bufs is None:
        x_bufs = nrt + 4

    with tile.TileContext(nc) as tc:
        with (
            tc.tile_pool(name="consts", bufs=1) as cpool,
            tc.tile_pool(name="xin", bufs=x_bufs) as xpool,
            tc.tile_pool(name="y1t", bufs=nwt + 4) as y1tpool,
            tc.tile_pool(name="outp", bufs=6) as outpool,
            tc.tile_pool(name="ps", bufs=ps_bufs, space="PSUM") as psp,
        ):
            store_engine = getattr(nc, store_eng)
            copy_i = 0

            def cb_copy(out, in_):
                nonlocal copy_i
                if copy_i % 4 < act_of_4:
                    nc.scalar.copy(out, in_)
                else:
                    nc.vector.tensor_copy(out, in_)
                copy_i += 1

            mtb_sb = cpool.tile([P, P], dt16)
            nc.sync.dma_start(mtb_sb[:], mtb_d[:])

            for img in range(n_img):
                r0 = img * img_h

                xts = []
                for rt in range(nrt):
                    xt = xpool.tile([P, width], dt16)
                    nc.sync.dma_start(
                        xt[:], x_d[r0 + rt * P : r0 + (rt + 1) * P, :]
                    )
                    xts.append(xt)

                # pass 1: y1t[wt][:, rt*128:+128] = (M @ x_chunk)^T
                y1ts = []
                for wt in range(nwt):
                    y1t = y1tpool.tile([P, img_h], dt16)
                    for g in range(nrt // mm_per_ps):
                        ps = psp.tile([P, psum_fd], F32)
                        for q in range(mm_per_ps):
                            rt = g * mm_per_ps + q
                            nc.tensor.matmul(
                                ps[:, q * P : (q + 1) * P],
                                xts[rt][:, wt * P : (wt + 1) * P],
                                mtb_sb[:],
                            )
                        cb_copy(
                            y1t[:, g * psum_fd : (g + 1) * psum_fd], ps[:]
                        )
                    y1ts.append(y1t)

                # pass 2: out[rt][:, wt*128:+128] = (M @ y1t_chunk)^T = final
                for rt in range(nrt):
                    out_sb = outpool.tile([P, width], dt16)
                    for g in range(nwt // mm_per_ps):
                        ps = psp.tile([P, psum_fd], F32)
                        for q in range(mm_per_ps):
                            wt = g * mm_per_ps + q
                            nc.tensor.matmul(
                                ps[:, q * P : (q + 1) * P],
                                y1ts[wt][:, rt * P : (rt + 1) * P],
                                mtb_sb[:],
                            )
                        cb_copy(
                            out_sb[:, g * psum_fd : (g + 1) * psum_fd], ps[:]
                        )
                    store_engine.dma_start(
                        out_d[r0 + rt * P : r0 + (rt + 1) * P, :], out_sb[:]
                    )

    nc.compile()
    return nc


def build_v4(n_img: int, img_h: int, width: int, dt16=FP16, psum_fd=1024,
             act_of_4=2, ps_bufs=4, x_bufs=20, y1t_bufs=18, out_bufs=6,
             warm_mm=48, load_gran=1, store_gran=1):
    """v3 + software pipelining: pass2 of image i-1 interleaves with pass1 of
    image i at PSUM-group granularity, so pass2 stationaries (y1t copies)
    have a full image-period of slack. A dummy-matmul warmup burst runs
    during the first image's load so HAM un-throttles before real work.
    """
    rows = n_img * img_h
    nrt, nwt = img_h // P, width // P
    mm_per_ps = psum_fd // P

    nc = bacc.Bacc("TRN2", target_bir_lowering=False, debug=False)
    x_d = nc.declare_dram_parameter("x", [rows, width], dt16, isOutput=False)
    mtb_d = nc.declare_dram_parameter("mtb", [P, P], dt16, isOutput=False)
    out_d = nc.declare_dram_parameter("out", [rows, width], dt16, isOutput=True)

    with tile.TileContext(nc) as tc:
        with (
            tc.tile_pool(name="consts", bufs=1) as cpool,
            tc.tile_pool(name="xin", bufs=x_bufs) as xpool,
            tc.tile_pool(name="y1t", bufs=y1t_bufs) as y1tpool,
            tc.tile_pool(name="outp", bufs=out_bufs) as outpool,
            tc.tile_pool(name="ps", bufs=ps_bufs, space="PSUM") as psp,
        ):
            copy_i = 0

            def cb_copy(out, in_):
                nonlocal copy_i
                if copy_i % 4 < act_of_4:
                    nc.scalar.copy(out, in_)
                else:
                    nc.vector.tensor_copy(out, in_)
                copy_i += 1

            mtb_sb = cpool.tile([P, P], dt16)
            nc.sync.dma_start(mtb_sb[:], mtb_d[:])

            xts_of = {}
            y1ts_of = {}

            def load_img(i):
                # granule tiles of load_gran row-tiles; per-partition runs
                # stay 2KB-contiguous, batched into one descriptor set
                r0 = i * img_h
                xg = []
                for g in range(nrt // load_gran):
                    xt = xpool.tile([P, load_gran, width], dt16)
                    src = x_d[
                        r0 + g * load_gran * P : r0 + (g + 1) * load_gran * P, :
                    ].rearrange("(t p) w -> p t w", p=P)
                    nc.sync.dma_start(xt[:], src)
                    xg.append(xt)
                xts_of[i] = xg

            def x_chunk(i, rt, wt):
                xt = xts_of[i][rt // load_gran]
                return xt[:, rt % load_gran, wt * P : (wt + 1) * P]

            def p1_group(i, wt):
                # y1t(i)[wt][:, rt*128:+128] = (M @ x_chunk)^T for all rt
                y1t = y1tpool.tile([P, img_h], dt16)
                for g in range(nrt // mm_per_ps):
                    ps = psp.tile([P, psum_fd], F32)
                    for q in range(mm_per_ps):
                        rt = g * mm_per_ps + q
                        nc.tensor.matmul(
                            ps[:, q * P : (q + 1) * P],
                            x_chunk(i, rt, wt),
                            mtb_sb[:],
                        )
                    cb_copy(y1t[:, g * psum_fd : (g + 1) * psum_fd], ps[:])
                y1ts_of.setdefault(i, []).append(y1t)

            out_cur = {}

            def p2_group(i, rt, half=None):
                # out(i)[rt][:, wt*128:+128] = (M @ y1t_chunk)^T
                # half=0/1: emit only the wt-half (4 matmuls + 1 copy); half 0
                # needs only y1t[0..3], so it can start before pass1 finishes
                r0 = i * img_h
                y1ts = y1ts_of[i]
                seg = rt % store_gran
                pair = rt - seg
                if pair not in out_cur:
                    out_cur[pair] = outpool.tile(
                        [P, store_gran, width], dt16, name="out_sb"
                    )
                out_sb = out_cur[pair]
                if half is None:
                    ps = psp.tile([P, psum_fd], F32)
                    for q in range(mm_per_ps):
                        nc.tensor.matmul(
                            ps[:, q * P : (q + 1) * P],
                            y1ts[q][:, rt * P : (rt + 1) * P],
                            mtb_sb[:],
                        )
                    cb_copy(out_sb[:, seg, :], ps[:])
                else:
                    ps = psp.tile([P, psum_fd], F32)
                    fd = psum_fd // 2
                    for q in range(fd // P):
                        wt = half * (fd // P) + q
                        nc.tensor.matmul(
                            ps[:, q * P : (q + 1) * P],
                            y1ts[wt][:, rt * P : (rt + 1) * P],
                            mtb_sb[:],
                        )
                    cb_copy(
                        out_sb[:, seg, half * fd : (half + 1) * fd], ps[:, :fd]
                    )
                if (half is None or half == 1) and seg == store_gran - 1:
                    del out_cur[pair]
                    dst = out_d[
                        r0 + pair * P : r0 + (pair + store_gran) * P, :
                    ].rearrange("(t p) w -> p t w", p=P)
                    nc.gpsimd.dma_start(dst, out_sb[:])

            # HAM warmup: dummy matmuls on the constant while image 0 loads.
            load_img(0)
            for w in range(warm_mm // mm_per_ps):
                ps = psp.tile([P, psum_fd], F32)
                for q in range(mm_per_ps):
                    nc.tensor.matmul(
                        ps[:, q * P : (q + 1) * P], mtb_sb[:], mtb_sb[:]
                    )

            for i in range(n_img):
                if i + 1 < n_img:
                    load_img(i + 1)
                for g in range(nwt):
                    p1_group(i, g)
                    if i > 0:
                        p2_group(i - 1, g)
            for g in range(nwt):
                p2_group(n_img - 1, g)

    nc.compile()
    return nc


def build_v7(n_img: int, img_h: int, width: int, dt16=FP16,
             x_bufs=6, y1t_bufs=18, out_bufs=8, warm_mm=48,
             load_gran=4, store_gran=2, copy_cycle="av", tail_cycle="av",
             last_store_gran=1, ps_bufs=8, psum_dt=None, flat=False):
    """v6 + finer (512-col) PSUM groups, and a last-image fast path: p2
    half0 of the final image interleaves with p1 g4..7 so the serial tail
    shrinks to the p2-half1 sweep + one small store.

    flat=True: DRAM x/out are host-permuted to [P, img*rowtile*width] so
    every load/store descriptor is one contiguous gran*2KB run per
    partition (vs scattered 2KB runs with the row-major layout).
    """
    rows = n_img * img_h
    nrt, nwt = img_h // P, width // P
    FD = 512
    HF = FD // P  # 4 matmuls per psum tile = half a row group
    PSDT = F32 if psum_dt is None else psum_dt

    nc = bacc.Bacc("TRN2", target_bir_lowering=False, debug=False)
    if flat:
        ppf = n_img * nrt * width  # per-partition free elems
        x_d = nc.declare_dram_parameter("x", [P, ppf], dt16, isOutput=False)
        out_d = nc.declare_dram_parameter("out", [P, ppf], dt16, isOutput=True)
    else:
        x_d = nc.declare_dram_parameter("x", [rows, width], dt16, isOutput=False)
        out_d = nc.declare_dram_parameter("out", [rows, width], dt16, isOutput=True)
    mtb_d = nc.declare_dram_parameter("mtb", [P, P], dt16, isOutput=False)

    with tile.TileContext(nc) as tc:
        with (
            tc.tile_pool(name="consts", bufs=1) as cpool,
            tc.tile_pool(name="xin", bufs=x_bufs) as xpool,
            tc.tile_pool(name="y1t", bufs=y1t_bufs) as y1tpool,
            tc.tile_pool(name="outp", bufs=out_bufs) as outpool,
            tc.tile_pool(name="ps", bufs=ps_bufs, space="PSUM") as psp,
        ):
            copy_i = 0

            def cb_copy(out, in_, cycle=copy_cycle):
                nonlocal copy_i
                c = cycle[copy_i % len(cycle)]
                if c == "a":
                    nc.scalar.copy(out, in_)
                elif c == "v":
                    nc.vector.tensor_copy(out, in_)
                else:
                    nc.gpsimd.tensor_copy(out, in_)
                copy_i += 1

            mtb_sb = cpool.tile([P, P], dt16)
            nc.sync.dma_start(mtb_sb[:], mtb_d[:])

            xts_of = {}
            y1ts_of = {}

            def load_img(i):
                r0 = i * img_h
                xg = []
                for g in range(nrt // load_gran):
                    xt = xpool.tile([P, load_gran, width], dt16)
                    src = x_d[
                        r0 + g * load_gran * P : r0 + (g + 1) * load_gran * P, :
                    ].rearrange("(t p) w -> p t w", p=P)
                    nc.sync.dma_start(xt[:], src)
                    xg.append(xt)
                xts_of[i] = xg

            def x_chunk(i, rt, wt):
                xt = xts_of[i][rt // load_gran]
                return xt[:, rt % load_gran, wt * P : (wt + 1) * P]

            def p1_half(i, wt, h):
                # y1t(i)[wt][:, h*512 : +512] = (M @ x rows h*512..)^T
                y1ts = y1ts_of.setdefault(i, {})
                if wt not in y1ts:
                    y1ts[wt] = y1tpool.tile([P, img_h], dt16, name="y1t")
                ps = psp.tile([P, FD], PSDT)
                for q in range(HF):
                    rt = h * HF + q
                    nc.tensor.matmul(
                        ps[:, q * P : (q + 1) * P],
                        x_chunk(i, rt, wt),
                        mtb_sb[:],
                    )
                cb_copy(y1ts[wt][:, h * FD : (h + 1) * FD], ps[:])

            out_cur = {}

            def p2_half(i, rt, h, gran=None, cycle=copy_cycle):
                # out(i)[rt][:, h*512 : +512] = (M @ y1t cols)^T
                gran = store_gran if gran is None else gran
                r0 = i * img_h
                y1ts = y1ts_of[i]
                seg = rt % gran
                pair = rt - seg
                if (i, pair) not in out_cur:
                    out_cur[(i, pair)] = outpool.tile(
                        [P, gran, width], dt16, name="out_sb"
                    )
                out_sb = out_cur[(i, pair)]
                ps = psp.tile([P, FD], PSDT)
                for q in range(HF):
                    wt = h * HF + q
                    nc.tensor.matmul(
                        ps[:, q * P : (q + 1) * P],
                        y1ts[wt][:, rt * P : (rt + 1) * P],
                        mtb_sb[:],
                    )
                cb_copy(out_sb[:, seg, h * FD : (h + 1) * FD], ps[:], cycle)
                if h == 1 and seg == gran - 1:
                    del out_cur[(i, pair)]
                    dst = out_d[
                        r0 + pair * P : r0 + (pair + gran) * P, :
                    ].rearrange("(t p) w -> p t w", p=P)
                    nc.gpsimd.dma_start(dst, out_sb[:])

            # HAM warmup: dummy matmuls on the constant while image 0 loads.
            load_img(0)
            for w in range(warm_mm // HF):
                ps = psp.tile([P, FD], PSDT)
                for q in range(HF):
                    nc.tensor.matmul(
                        ps[:, q * P : (q + 1) * P], mtb_sb[:], mtb_sb[:]
                    )

            # image 0: sweep h0 over all groups first (needs only granule 0)
            for g in range(nwt):
                p1_half(0, g, 0)
            for g in range(nwt):
                p1_half(0, g, 1)

            for i in range(1, n_img):
                load_img(i)
                last = i == n_img - 1
                for g in range(nwt):
                    p1_half(i, g, 0)
                    p1_half(i, g, 1)
                    p2_half(i - 1, g, 0)
                    p2_half(i - 1, g, 1)
                    if last and g >= nwt // 2:
                        # y1t[i][0..3] complete after g=3 -> start p2 half0
                        rt = 2 * (g - nwt // 2)
                        p2_half(i, rt, 0, gran=last_store_gran,
                                cycle=tail_cycle)
                        p2_half(i, rt + 1, 0, gran=last_store_gran,
                                cycle=tail_cycle)
            for rt in range(nrt):
                p2_half(n_img - 1, rt, 1, gran=last_store_gran,
                        cycle=tail_cycle)

    nc.compile()
    return nc


def build_v9(n_img: int, img_h: int, width: int, dt16=FP16, FD=1024,
             x_bufs=8, y1t_bufs=18, out_bufs=20, warm_mm=48,
             load_gran=4, store_gran=2, copy_cycle="av", tail_cycle="av",
             ps_bufs=4, ramp_split=True, p2_first=False, tail_gran=1,
             tail_bufs=6, first_load_gran=None, warm_memset=False):
    """Flat per-partition DRAM layout ([P, img*rowtile*width], host permutes)
    so every DMA descriptor is one contiguous multi-KB run per partition.
    Deep out pool: compute runs ahead of stores, the tail drains backlog at
    full DMA rate. FD-wide PSUM groups amortize the ~310ns copy overhead.
    Last image: p2 runs in 512-col half groups interleaved with p1 g4..7.
    """
    rows = n_img * img_h
    nrt, nwt = img_h // P, width // P
    HF = FD // P
    NG = img_h // FD  # psum groups per (pass, column-tile)

    nc = bacc.Bacc("TRN2", target_bir_lowering=False, debug=False)
    ppf = n_img * nrt * width
    x_d = nc.declare_dram_parameter("x", [P, ppf], dt16, isOutput=False)
    out_d = nc.declare_dram_parameter("out", [P, ppf], dt16, isOutput=True)
    mtb_d = nc.declare_dram_parameter("mtb", [P, P], dt16, isOutput=False)

    with tile.TileContext(nc) as tc:
        with (
            tc.tile_pool(name="consts", bufs=1) as cpool,
            tc.tile_pool(name="xin", bufs=x_bufs) as xpool,
            tc.tile_pool(name="y1t", bufs=y1t_bufs) as y1tpool,
            tc.tile_pool(name="outp", bufs=out_bufs) as outpool,
            tc.tile_pool(name="tailo", bufs=tail_bufs) as tailpool,
            tc.tile_pool(name="ps", bufs=ps_bufs, space="PSUM") as psp,
        ):
            copy_i = 0

            def cb_copy(out, in_, cycle=copy_cycle):
                nonlocal copy_i
                c = cycle[copy_i % len(cycle)]
                if c == "a":
                    nc.scalar.copy(out, in_)
                else:
                    nc.vector.tensor_copy(out, in_)
                copy_i += 1

            mtb_sb = cpool.tile([P, P], dt16)
            if not warm_memset:
                nc.sync.dma_start(mtb_sb[:], mtb_d[:])

            xts_of = {}
            y1ts_of = {}

            gran_of = {}

            def load_img(i, gran=None):
                gran = gran or load_gran
                gran_of[i] = gran
                xg = []
                for g in range(nrt // gran):
                    xt = xpool.tile([P, gran * width], dt16)
                    off = (i * nrt + g * gran) * width
                    nc.sync.dma_start(xt[:], x_d[:, off : off + gran * width])
                    xg.append(xt)
                xts_of[i] = xg

            def x_chunk(i, rt, wt):
                gran = gran_of[i]
                xt = xts_of[i][rt // gran]
                return xt[:, (rt % gran) * width + wt * P :
                          (rt % gran) * width + (wt + 1) * P]

            def get_y1t(i, wt):
                y1ts = y1ts_of.setdefault(i, {})
                if wt not in y1ts:
                    y1ts[wt] = y1tpool.tile([P, img_h], dt16, name="y1t")
                return y1ts[wt]

            def p1_group(i, wt, rt0, nmm, cycle=copy_cycle):
                # y1t(i)[wt][:, rt0*P..] = (M @ x rows rt0..)^T, nmm matmuls
                y1t = get_y1t(i, wt)
                ps = psp.tile([P, FD], F32)
                for q in range(nmm):
                    nc.tensor.matmul(
                        ps[:, q * P : (q + 1) * P],
                        x_chunk(i, rt0 + q, wt),
                        mtb_sb[:],
                    )
                cb_copy(
                    y1t[:, rt0 * P : (rt0 + nmm) * P], ps[:, : nmm * P], cycle
                )

            out_cur = {}

            def p2_group(i, rt, wt0, nmm, gran, cycle=copy_cycle):
                # out(i)[rt][:, wt0*P..] = (M @ y1t cols)^T, nmm matmuls
                y1ts = y1ts_of[i]
                seg = rt % gran
                pair = rt - seg
                if (i, pair) not in out_cur:
                    pool = outpool if gran == store_gran else tailpool
                    out_cur[(i, pair)] = pool.tile(
                        [P, gran * width], dt16, name="out_sb"
                    )
                out_sb = out_cur[(i, pair)]
                ps = psp.tile([P, FD], F32)
                for q in range(nmm):
                    nc.tensor.matmul(
                        ps[:, q * P : (q + 1) * P],
                        y1ts[wt0 + q][:, rt * P : (rt + 1) * P],
                        mtb_sb[:],
                    )
                cb_copy(
                    out_sb[:, seg * width + wt0 * P : seg * width + (wt0 + nmm) * P],
                    ps[:, : nmm * P],
                    cycle,
                )
                if wt0 + nmm == nwt and seg == gran - 1:
                    del out_cur[(i, pair)]
                    off = (i * nrt + pair) * width
                    nc.gpsimd.dma_start(
                        out_d[:, off : off + gran * width], out_sb[:]
                    )

            # HAM warmup: dummy matmuls while image 0 loads. With
            # warm_memset, x granule 0 posts FIRST on the sync ring (the
            # 256B-descriptor mtb load would delay it), and the warmup
            # runs on a memset tile so it does not wait for mtb either.
            fg = first_load_gran or load_gran
            if warm_memset:
                warm_sb = cpool.tile([P, P], dt16)
                load_img(0, fg)
                nc.sync.dma_start(mtb_sb[:], mtb_d[:])
                nc.vector.memset(warm_sb[:], 0.25)
            else:
                warm_sb = mtb_sb
                load_img(0, fg)
            for w in range(warm_mm // HF):
                ps = psp.tile([P, FD], F32)
                for q in range(HF):
                    nc.tensor.matmul(
                        ps[:, q * P : (q + 1) * P], warm_sb[:], warm_sb[:]
                    )

            # image 0 ramp: half-image sweeps so p1 starts at granule 0
            if ramp_split and fg * 2 == nrt:
                for g in range(nwt):
                    p1_group(0, g, 0, nrt // 2)
                for g in range(nwt):
                    p1_group(0, g, nrt // 2, nrt // 2)
            else:
                for g in range(nwt):
                    for gg in range(NG):
                        p1_group(0, g, gg * HF, HF)

            for i in range(1, n_img):
                load_img(i)
                last = i == n_img - 1
                if p2_first and not last:
                    # in-order PE: put guaranteed-ready p2 work ahead of
                    # p1 matmuls that may wait on the image-i load
                    for g in range(nwt):
                        for gg in range(NG):
                            p2_group(i - 1, g, gg * HF, HF, store_gran)
                    for g in range(nwt):
                        for gg in range(NG):
                            p1_group(i, g, gg * HF, HF)
                    continue
                for g in range(nwt):
                    for gg in range(NG):
                        p1_group(i, g, gg * HF, HF)
                    for gg in range(NG):
                        p2_group(i - 1, g, gg * HF, HF, store_gran)
                    if last and g >= nwt // 2:
                        # y1t[i][0..3] done after g=3 -> p2 half0 (4 mms)
                        rt = 2 * (g - nwt // 2)
                        p2_group(i, rt, 0, 4, tail_gran, cycle=tail_cycle)
                        p2_group(i, rt + 1, 0, 4, tail_gran, cycle=tail_cycle)
            # h1 sweep: rt 0..3 have h0 done already; each h1 fires its store
            li = n_img - 1
            for rt in range(nrt):
                p2_group(li, rt, 4, 4, tail_gran, cycle=tail_cycle)

    nc.compile()
    return nc


def build_v10(n_img: int, img_h: int, width: int, dt16=FP16,
              x_bufs=8, y1t_bufs=3, out_bufs=19, warm_mm=48,
              load_gran=4, copy_cycle="av", tail_cycle="av", ps_bufs=2,
              p2_first=False):
    """v9 + 2048-col PSUM groups (4 banks x 2 bufs) with one-tile-per-image
    y1t, so one copy spans a wt-pair (p1) or an rt-pair (p2, firing a full
    0.5MB store). 64 copies of ~1.8us replace 144 of ~1us: the ~310ns fixed
    copy overhead amortizes and ACT/DVE stop pacing the pipeline.
    """
    rows = n_img * img_h
    nrt, nwt = img_h // P, width // P
    FD = 2048

    nc = bacc.Bacc("TRN2", target_bir_lowering=False, debug=False)
    ppf = n_img * nrt * width
    x_d = nc.declare_dram_parameter("x", [P, ppf], dt16, isOutput=False)
    out_d = nc.declare_dram_parameter("out", [P, ppf], dt16, isOutput=True)
    mtb_d = nc.declare_dram_parameter("mtb", [P, P], dt16, isOutput=False)

    with tile.TileContext(nc) as tc:
        with (
            tc.tile_pool(name="consts", bufs=1) as cpool,
            tc.tile_pool(name="xin", bufs=x_bufs) as xpool,
            tc.tile_pool(name="y1t", bufs=y1t_bufs) as y1tpool,
            tc.tile_pool(name="outp", bufs=out_bufs) as outpool,
            tc.tile_pool(name="tailp", bufs=8) as tailpool,
            tc.tile_pool(name="ps", bufs=ps_bufs, space="PSUM") as psp,
        ):
            copy_i = 0

            def cb_copy(out, in_, cycle=copy_cycle):
                nonlocal copy_i
                c = cycle[copy_i % len(cycle)]
                if c == "a":
                    nc.scalar.copy(out, in_)
                else:
                    nc.vector.tensor_copy(out, in_)
                copy_i += 1

            mtb_sb = cpool.tile([P, P], dt16)
            nc.sync.dma_start(mtb_sb[:], mtb_d[:])

            xts_of = {}
            y1t_of = {}

            def load_img(i):
                xg = []
                for g in range(nrt // load_gran):
                    xt = xpool.tile([P, load_gran * width], dt16)
                    off = (i * nrt + g * load_gran) * width
                    nc.sync.dma_start(xt[:], x_d[:, off : off + load_gran * width])
                    xg.append(xt)
                xts_of[i] = xg

            def x_chunk(i, rt, wt):
                xt = xts_of[i][rt // load_gran]
                return xt[:, (rt % load_gran) * width + wt * P :
                          (rt % load_gran) * width + (wt + 1) * P]

            def get_y1t(i):
                if i not in y1t_of:
                    y1t_of[i] = y1tpool.tile([P, nwt * img_h], dt16, name="y1t")
                return y1t_of[i]

            def p1_pair(i, w2):
                # y1t(i)[:, w2*1024 : +2048] = (M @ x)^T for wt w2, w2+1
                y1t = get_y1t(i)
                ps = psp.tile([P, FD], F32)
                for q in range(16):
                    wt, rt = w2 + q // 8, q % 8
                    nc.tensor.matmul(
                        ps[:, q * P : (q + 1) * P],
                        x_chunk(i, rt, wt),
                        mtb_sb[:],
                    )
                cb_copy(y1t[:, w2 * img_h : (w2 + 2) * img_h], ps[:])

            def p1_pair_half(i, w2, rt0):
                # ramp variant: rt0..rt0+3 only; two 512-col copies
                y1t = get_y1t(i)
                ps = psp.tile([P, FD], F32)
                for q in range(8):
                    wt, rt = w2 + q // 4, rt0 + q % 4
                    nc.tensor.matmul(
                        ps[:, q * P : (q + 1) * P],
                        x_chunk(i, rt, wt),
                        mtb_sb[:],
                    )
                cb_copy(y1t[:, w2 * img_h + rt0 * P : w2 * img_h + (rt0 + 4) * P],
                        ps[:, :512])
                cb_copy(
                    y1t[:, (w2 + 1) * img_h + rt0 * P :
                        (w2 + 1) * img_h + (rt0 + 4) * P],
                    ps[:, 512:1024],
                )

            def p2_pair(i, r2):
                # out rows r2, r2+1 complete -> store 0.5MB
                y1t = get_y1t(i)
                ps = psp.tile([P, FD], F32)
                out_sb = outpool.tile([P, FD], dt16, name="out_sb")
                for q in range(16):
                    seg, wt = q // 8, q % 8
                    nc.tensor.matmul(
                        ps[:, q * P : (q + 1) * P],
                        y1t[:, wt * img_h + (r2 + seg) * P :
                            wt * img_h + (r2 + seg + 1) * P],
                        mtb_sb[:],
                    )
                cb_copy(out_sb[:], ps[:])
                off = (i * nrt + r2) * width
                nc.gpsimd.dma_start(out_d[:, off : off + FD], out_sb[:])

            tail_out = {}

            def p2_tail(i, rt, h, cycle=tail_cycle):
                # gran-1 half groups for the last image
                y1t = get_y1t(i)
                if rt not in tail_out:
                    tail_out[rt] = tailpool.tile([P, width], dt16, name="out_sb1")
                out_sb = tail_out[rt]
                ps = psp.tile([P, FD], F32)
                for q in range(4):
                    wt = h * 4 + q
                    nc.tensor.matmul(
                        ps[:, q * P : (q + 1) * P],
                        y1t[:, wt * img_h + rt * P : wt * img_h + (rt + 1) * P],
                        mtb_sb[:],
                    )
                cb_copy(out_sb[:, h * 512 : (h + 1) * 512], ps[:, :512], cycle)
                if h == 1:
                    off = (i * nrt + rt) * width
                    nc.gpsimd.dma_start(out_d[:, off : off + width], out_sb[:])

            # HAM warmup while image 0 loads
            load_img(0)
            for w in range(warm_mm // 16):
                ps = psp.tile([P, FD], F32)
                for q in range(16):
                    nc.tensor.matmul(
                        ps[:, q * P : (q + 1) * P], mtb_sb[:], mtb_sb[:]
                    )

            # image 0 ramp: rt-half sweeps so p1 starts on granule 0
            for w2 in range(0, nwt, 2):
                p1_pair_half(0, w2, 0)
            for w2 in range(0, nwt, 2):
                p1_pair_half(0, w2, 4)

            for i in range(1, n_img):
                load_img(i)
                if i < n_img - 1:
                    if p2_first:
                        for w2 in range(0, nwt, 2):
                            p2_pair(i - 1, w2)
                        for w2 in range(0, nwt, 2):
                            p1_pair(i, w2)
                        continue
                    for w2 in range(0, nwt, 2):
                        p1_pair(i, w2)
                        p2_pair(i - 1, w2)
                else:
                    for w2 in (0, 2):
                        p1_pair(i, w2)
                        p2_pair(i - 1, w2)
                    # y1t(i) wt 0..3 ready: start last-image p2 half0s
                    p1_pair(i, 4)
                    p2_pair(i - 1, 4)
                    for rt in range(0, 4):
                        p2_tail(i, rt, 0)
                    p1_pair(i, 6)
                    p2_pair(i - 1, 6)
                    for rt in range(4, nrt):
                        p2_tail(i, rt, 0)
            for rt in range(nrt):
                p2_tail(n_img - 1, rt, 1)

    nc.compile()
    return nc


def build_hybrid(n_img: int, img_h: int, width: int):
    rows = n_img * img_h
    nrt, nwt = img_h // P, width // P
    assert nrt % 4 == 0 and nwt % 4 == 0
    MMW = 512

    nc = bacc.Bacc("TRN2", target_bir_lowering=False, debug=False)
    x_d = nc.declare_dram_parameter("x", [rows, width], F32R, isOutput=False)
    mt_d = nc.declare_dram_parameter("mt", [P, P], F32R, isOutput=False)
    mtb_d = nc.declare_dram_parameter("mtb", [P, P], BF16, isOutput=False)
    identb_d = nc.declare_dram_parameter("identb", [P, P], BF16, isOutput=False)
    out_d = nc.declare_dram_parameter("out", [rows, width], F32, isOutput=True)

    with tile.TileContext(nc) as tc:
        with (
            tc.tile_pool(name="consts", bufs=1) as cpool,
            tc.tile_pool(name="xin", bufs=6) as xpool,
            tc.tile_pool(name="y1", bufs=nrt + 2) as y1pool,
            tc.tile_pool(name="y1t", bufs=nwt + 4) as y1tpool,
            tc.tile_pool(name="outp", bufs=4) as outpool,
            tc.tile_pool(name="psV", bufs=3, space="PSUM") as psV,
            tc.tile_pool(name="psT", bufs=3, space="PSUM") as psT,
            tc.tile_pool(name="psH", bufs=2, space="PSUM") as psH,
        ):
            cb = _CopyBalancer(nc)
            mt_sb = cpool.tile([P, P], F32R)
            mtb_sb = cpool.tile([P, P], BF16)
            identb = cpool.tile([P, P], BF16)
            nc.sync.dma_start(mt_sb[:], mt_d[:])
            nc.sync.dma_start(mtb_sb[:], mtb_d[:])
            nc.sync.dma_start(identb[:], identb_d[:])

            for img in range(n_img):
                r0 = img * img_h

                # V-pass: f32r stream, round to bf16 on the PSUM->SBUF copy
                y1s = []
                for rt in range(nrt):
                    xt = xpool.tile([P, width], F32R)
                    nc.sync.dma_start(
                        xt[:], x_d[r0 + rt * P : r0 + (rt + 1) * P, :]
                    )
                    y1 = y1pool.tile([P, width], BF16)
                    for c in range(width // MMW):
                        ps = psV.tile([P, MMW], F32)
                        nc.tensor.matmul(
                            ps[:], mt_sb[:], xt[:, c * MMW : (c + 1) * MMW]
                        )
                        cb.copy(y1[:, c * MMW : (c + 1) * MMW], ps[:])
                    y1s.append(y1)

                # T-pass: bf16 PE transposes, 8 per PSUM bank
                y1ts = []
                for wt in range(nwt):
                    y1t = y1tpool.tile([P, img_h], BF16)
                    pst = psT.tile([P, img_h], BF16)
                    for rt in range(nrt):
                        nc.tensor.transpose(
                            pst[:, rt * P : (rt + 1) * P],
                            y1s[rt][:, wt * P : (wt + 1) * P],
                            identb[:],
                        )
                    cb.copy(y1t[:], pst[:])
                    y1ts.append(y1t)

                # fused H-pass: out chunk = (y1t_chunk)^T @ M^T  (H-major)
                for rt in range(nrt):
                    out_sb = outpool.tile([P, width], F32)
                    for half in range(nwt // 4):
                        ps = psH.tile([P, 512], F32)
                        for q in range(4):
                            wt = half * 4 + q
                            nc.tensor.matmul(
                                ps[:, q * P : (q + 1) * P],
                                y1ts[wt][:, rt * P : (rt + 1) * P],
                                mtb_sb[:],
                            )
                        cb.copy(out_sb[:, half * 512 : (half + 1) * 512], ps[:])
                    nc.sync.dma_start(
                        out_d[r0 + rt * P : r0 + (rt + 1) * P, :], out_sb[:]
                    )

    nc.compile()
    return nc


def build_nc(n_img, img_h, width, mode=MODE):
    if mode == "fused_bf16":
        return build_fused_bf16(n_img, img_h, width, BF16)
    if mode == "fused_fp16":
        return build_fused_bf16(n_img, img_h, width, FP16)
    if mode == "hybrid":
        return build_hybrid(n_img, img_h, width)
    if mode == "v2":
        return build_v2(n_img, img_h, width)
    if mode == "v3":
        return build_v2(n_img, img_h, width, psum_fd=1024, ps_bufs=4,
                        store_eng="gpsimd", x_bufs=20)
    if mode == "v4":
        return build_v4(n_img, img_h, width)
    if mode == "v6":
        return build_v4(n_img, img_h, width, load_gran=4, store_gran=2,
                        x_bufs=5, out_bufs=4)
    if mode == "v6b":
        return build_v4(n_img, img_h, width, load_gran=4, store_gran=4,
                        x_bufs=5, out_bufs=3)
    if mode == "v7":
        return build_v7(n_img, img_h, width)
    if mode == "v7noG":
        return build_v7(n_img, img_h, width, copy_cycle="av")
    if mode == "v9":
        return build_v9(n_img, img_h, width)
    if mode == "v9fd512":
        return build_v9(n_img, img_h, width, FD=512, ps_bufs=8)
    if mode == "v9shallow":
        return build_v9(n_img, img_h, width, x_bufs=5, out_bufs=6)
    if mode == "v9b":
        return build_v9(n_img, img_h, width, p2_first=True)
    if mode == "v10":
        return build_v10(n_img, img_h, width)
    if mode == "v10lg8":
        return build_v10(n_img, img_h, width, load_gran=8, x_bufs=4)
    if mode == "v10p2f":
        return build_v10(n_img, img_h, width, p2_first=True)
    if mode == "v10w32":
        return build_v10(n_img, img_h, width, warm_mm=32, p2_first=True)
    if mode == "v11":
        return build_v9(n_img, img_h, width, out_bufs=25, warm_mm=32,
                        copy_cycle="avavavavavavavava")
    if mode == "v11x10":
        return build_v9(n_img, img_h, width, x_bufs=10, out_bufs=22,
                        warm_mm=32, copy_cycle="avavavavavavavava")
    if mode == "v12":
        return build_v9(n_img, img_h, width, store_gran=1, out_bufs=50,
                        warm_mm=32, copy_cycle="avavavavavavavava")
    if mode == "v12x6":
        return build_v9(n_img, img_h, width, store_gran=1, x_bufs=6,
                        out_bufs=58, warm_mm=32,
                        copy_cycle="avavavavavavavava")
    if mode == "v13":
        return build_v9(n_img, img_h, width, store_gran=4, out_bufs=11,
                        tail_gran=2, tail_bufs=4, warm_mm=32)
    if mode == "v13lg8":
        return build_v9(n_img, img_h, width, store_gran=4, out_bufs=11,
                        tail_gran=2, tail_bufs=4, warm_mm=32,
                        load_gran=8, x_bufs=4)
    if mode == "v14":
        return build_v9(n_img, img_h, width, store_gran=4, out_bufs=11,
                        tail_gran=2, tail_bufs=4, warm_mm=32,
                        load_gran=8, x_bufs=4, first_load_gran=4)
    if mode == "v14sg8":
        return build_v9(n_img, img_h, width, store_gran=8, out_bufs=5,
                        tail_gran=2, tail_bufs=6, warm_mm=32,
                        load_gran=8, x_bufs=4, first_load_gran=4)
    if mode == "v15":
        return build_v9(n_img, img_h, width, store_gran=4, out_bufs=11,
                        tail_gran=2, tail_bufs=4, warm_mm=32,
                        load_gran=8, x_bufs=4, first_load_gran=4,
                        warm_memset=True)
    raise ValueError(mode)


def make_mt(A: np.ndarray) -> np.ndarray:
    """M^T where M = kron(I_{128/8}, A)."""
    M = np.kron(np.eye(P // BLOCK, dtype=np.float32), A.astype(np.float32))
    return np.ascontiguousarray(M.T)


def is_flat(mode):
    return mode.startswith("v9") or mode.startswith("v1")


def unpack_out(mode, arr, per, C, H, W):
    """Device 'out' array -> [per, C, H, W] float32 view of this core."""
    if is_flat(mode):
        n_img, nrt = per * C, H // P
        a = arr.reshape(P, n_img, nrt, W).transpose(1, 2, 0, 3)
        return np.ascontiguousarray(a).reshape(per, C, H, W)
    return arr.reshape(per, C, H, W)


def make_inputs(mode, x_core, A):
    mt = make_mt(A)
    if mode == "fused_bf16":
        return {"x": x_core, "mtb": mt.astype(ml_dtypes.bfloat16)}
    if mode == "fused_fp16":
        return {"x": x_core, "mtb": mt.astype(np.float16)}
    if is_flat(mode):
        rows, width = x_core.shape
        nrt = FULL_H // P
        n_img = rows // FULL_H
        xp = (
            x_core.astype(np.float16)
            .reshape(n_img, nrt, P, width)
            .transpose(2, 0, 1, 3)
        )
        return {
            "x": np.ascontiguousarray(xp).reshape(P, -1),
            "mtb": mt.astype(np.float16),
        }
    if mode[0] == "v" and mode[1].isdigit() and int(mode[1]) >= 2:
        return {"x": x_core.astype(np.float16), "mtb": mt.astype(np.float16)}
    if mode == "hybrid":
        return {
            "x": x_core,
            "mt": mt,
            "mtb": mt.astype(ml_dtypes.bfloat16),
            "identb": np.eye(P, dtype=ml_dtypes.bfloat16),
        }
    raise ValueError(mode)


_NC_CACHE = {}


def _get_nc(key, *args, **kwargs):
    if key not in _NC_CACHE:
        _NC_CACHE[key] = build_nc(*args, **kwargs)
    return _NC_CACHE[key]


def kernel(x: np.ndarray, A: np.ndarray) -> np.ndarray:
    x = np.asarray(x, dtype=np.float32)
    A = np.asarray(A, dtype=np.float32)
    N, C, H, W = x.shape
    assert (N, C, H, W) == (FULL_N, FULL_C, FULL_H, FULL_W), x.shape
    per = N // N_CORES

    nc = _get_nc(("full", MODE), per * C, H, W, MODE)

    in_maps = [
        make_inputs(
            MODE,
            np.ascontiguousarray(x[c * per : (c + 1) * per].reshape(per * C * H, W)),
            A,
        )
        for c in range(N_CORES)
    ]
    def dc_ok(outs):
        # DC coeff of block (0,0) must equal mean*8 of the 8x8 input block;
        # catches transient device faults that corrupt output silently
        for n in range(0, N, 7):
            dc = float(x[n, 0, :8, :8].sum()) / 8.0
            if abs(float(outs[n // per][n % per, 0, 0, 0]) - dc) > 0.05 + 0.02 * abs(dc):
                return False
        return True

    last_err = None
    for _attempt in range(3):
        try:
            res = run_bass_kernel_spmd(nc, in_maps, list(range(N_CORES)))
        except Exception as e:  # transient NRT device faults: retry
            last_err = e
            continue
        outs = [
            unpack_out(
                MODE,
                res.results[c]["out"].astype(np.float32, copy=False),
                per, C, H, W,
            )
            for c in range(N_CORES)
        ]
        if dc_ok(outs):
            return np.concatenate(outs, axis=0)
        last_err = RuntimeError("DC self-check failed (corrupt output)")
    raise last_err

